# revision 26
# baseline (speedup 1.0000x reference)
"""NerfExperts MoE kernel for Trainium2, expert-parallel over 8 NeuronCores.

Strategy: each of the 1024 points is routed to one of 100 experts
(~2.3MB of fp32 weights each, ~232MB total -> memory bound).  We shard
the *experts* across the 8 cores (13 slots per core), dispatch tokens to
their expert's core on the host, and stream each expert's weights from
HBM exactly once, as bf16.  Weights are streamed LAYER-MAJOR (one DMA
chunk per layer covering all local experts, balanced across the two
HWDGE rings) so compute for layer l only waits on chunk l and the
DMA/compute pipeline drains with a short tail.  Short slabs (the 53-row
embedding-consuming ones) are packed two experts per 128 partitions.
Activations stay transposed ([feature, token]); experts advance through
the MLP in lockstep "waves" that share PSUM tiles, so PSUM->SBUF
bias+activation moves are batched across a wave (per-expert fp32 biases
via stride-0 broadcast APs on DVE, relu on ACT).  Harmonic-embedding
phases are computed in fp32 with Cody-Waite range reduction for Sin.

Embedding tile rows: points: sin 0:18, cos 32:50, xyz 50:53 (53 rows);
dirs: sin 0:12, cos 32:44, xyz 44:47 (47 rows).  Dead rows are zero in
the weight slabs, so garbage there is harmless; paired slabs use row
base 64 for the odd expert of each pair.
"""

import numpy as np
import ml_dtypes

import concourse.bass as bass
import concourse.bacc as bacc
import concourse.mybir as mybir
import concourse.tile as tile
from concourse.bass_utils import run_bass_kernel_spmd

PI = float(np.pi)
N_CORES = 8
E = 100
NX, ND = 6, 4
CAP_MAX = 128  # max tokens per expert slot (keeps matmul N and PSUM in range)

# Weight chunks (key = mlp stage fed; "P" carries the paired short slabs:
# L0, L5-skip, wc0-rays; stage 9 (wa) rides in chunk 8, wc1 in chunk 10).
CHUNKS = ["P", 1, 2, 3, 4, 5, 6, 7, 8, 10]
CHUNK_COLS = {1: 512, 2: 512, 3: 512, 4: 512, 5: 512,
              6: 512, 7: 512, 8: 514, 10: 259}
P_PAIR_COLS = 640          # per expert PAIR: L0 256 + L5skip 256 + rays 128

# fp32 bias tensor [128, 21*nslot], layer-major columns:
#   mlp stage lidx in 0..8 (layers 0-7, then wi): col = lidx*2*nslot + s*2 + j
#   ba: 18*nslot + s ; bc0: 19*nslot + s ; bc1: 20*nslot + s
NB = 21
PAIRED = False  # base-64 paired slabs hang real HW (sim-clean); keep full-height slabs


def _chunk_offsets(nslot):
    npair = (nslot + 1) // 2 if PAIRED else nslot
    coff, tot = {}, 0
    coff["P"] = 0
    tot += npair * P_PAIR_COLS
    for st in CHUNKS[1:]:
        coff[st] = tot
        tot += nslot * CHUNK_COLS[st]
    return coff, tot


def _pack_expert(wt, bt, s, nslot, inputs, e, coff):
    """Fill slot s columns of the per-stage blocks of wt [128, TOT] (fp32 view)
    and bias columns of bt [128, 21*nslot]."""
    n2 = 2 * nslot
    npair = (nslot + 1) // 2 if PAIRED else nslot
    rb = 64 * (s % 2) if PAIRED else 0   # row base in paired chunk
    pair = s // 2 if PAIRED else s

    def set_b2(lidx, b):
        bt[:, lidx * n2 + s * 2] = b[0:128]
        bt[:, lidx * n2 + s * 2 + 1] = b[128:256]

    # --- paired chunk P ---
    o = coff["P"] + pair * 256
    w0 = inputs["w0"][e]                             # [39, 256]
    wt[rb + 0: rb + 18, o: o + 256] = w0[0:18]       # sin
    wt[rb + 32: rb + 50, o: o + 256] = w0[18:36]     # cos
    wt[rb + 50: rb + 53, o: o + 256] = w0[36:39]     # xyz
    set_b2(0, inputs["b0"][e])
    o = coff["P"] + npair * 256 + pair * 256
    w5 = inputs["w5"][e]                             # [295, 256]
    wt[rb + 0: rb + 18, o: o + 256] = w5[256:274]
    wt[rb + 32: rb + 50, o: o + 256] = w5[274:292]
    wt[rb + 50: rb + 53, o: o + 256] = w5[292:295]
    o = coff["P"] + npair * 512 + pair * 128
    wc0 = inputs["wc0"][e]                           # [283, 128]
    wt[rb + 0: rb + 12, o: o + 128] = wc0[256:268]
    wt[rb + 32: rb + 44, o: o + 128] = wc0[268:280]
    wt[rb + 44: rb + 47, o: o + 128] = wc0[280:283]
    # --- full-height chunks ---
    for l in (1, 2, 3, 4, 6, 7):
        w = inputs[f"w{l}"][e]                       # [256, 256]
        o = coff[l] + s * 512
        for k in (0, 1):
            wt[:, o + k * 256: o + (k + 1) * 256] = w[128 * k: 128 * (k + 1)]
        set_b2(l, inputs[f"b{l}"][e])
    o = coff[5] + s * 512
    for k in (0, 1):
        wt[:, o + k * 256: o + (k + 1) * 256] = w5[128 * k: 128 * (k + 1)]
    set_b2(5, inputs["b5"][e])
    o = coff[8] + s * 514
    wi = inputs["wi"][e]
    for k in (0, 1):
        wt[:, o + k * 256: o + (k + 1) * 256] = wi[128 * k: 128 * (k + 1)]
    set_b2(8, inputs["bi"][e])
    wa = inputs["wa"][e][:, 0]                       # [256]
    wt[:, o + 512] = wa[0:128]
    wt[:, o + 513] = wa[128:256]
    bt[0, 18 * nslot + s] = inputs["ba"][e][0]
    o = coff[10] + s * 259
    wt[:, o: o + 128] = wc0[0:128]
    wt[:, o + 128: o + 256] = wc0[128:256]
    bt[:, 19 * nslot + s] = inputs["bc0"][e]
    wt[:, o + 256: o + 259] = inputs["wc1"][e]
    bt[0:3, 20 * nslot + s] = inputs["bc1"][e]


def _make_waves(nslot, C):
    gmax = max(1, min(512 // (2 * C), 6))
    nw = int(np.ceil(nslot / gmax))
    base = nslot // nw
    rem = nslot - base * nw
    sizes = [base + (1 if i < rem else 0) for i in range(nw)]
    waves, s0 = [], 0
    for g in sizes:
        waves.append((s0, s0 + g))
        s0 += g
    return waves


# ---------------------------------------------------------------------------
# Device program
# ---------------------------------------------------------------------------

def _build_program(C, nslot):
    """Build the SPMD Bass program: nslot expert slots of C tokens each."""
    nall = nslot * C
    npair = (nslot + 1) // 2 if PAIRED else nslot
    waves = _make_waves(nslot, C)
    nw = len(waves)
    coff, totcols = _chunk_offsets(nslot)
    f32 = mybir.dt.float32
    bf16 = mybir.dt.bfloat16
    Sin = mybir.ActivationFunctionType.Sin
    Sigmoid = mybir.ActivationFunctionType.Sigmoid
    Relu = mybir.ActivationFunctionType.Relu
    ADD = mybir.AluOpType.add
    SUB = mybir.AluOpType.subtract
    MUL = mybir.AluOpType.mult
    MAX = mybir.AluOpType.max
    MIN = mybir.AluOpType.min
    # range-reduction constants (Cody-Waite, fp32 magic rounding)
    INV2PI = float(np.float32(1.0 / (2 * PI)))
    MAGIC = 12582912.0            # 1.5 * 2**23: forces round-to-int in fp32
    C1 = 6.28125                  # 2*pi high part, exact in fp32
    C2 = float(np.float32(2 * PI - 6.28125))
    CLAMP = 3.1415925             # just under pi (ACT Sin domain is [-pi, pi])
    HALF_PI = float(np.float32(PI / 2))

    nc = bacc.Bacc("TRN2", target_bir_lowering=False, debug=False)
    wt_d = nc.dram_tensor("wt", (128, totcols), bf16, kind="ExternalInput")
    bt_d = nc.dram_tensor("bt", (128, NB * nslot), f32, kind="ExternalInput")
    pts_d = nc.dram_tensor("ptsT", (3, nall), f32, kind="ExternalInput")
    dir_d = nc.dram_tensor("dirT", (3, nall), f32, kind="ExternalInput")
    fx_d = nc.dram_tensor("fx", (3, 18), f32, kind="ExternalInput")
    fd_d = nc.dram_tensor("fd", (3, 12), f32, kind="ExternalInput")
    xyz_d = nc.dram_tensor("xyzb", (6, nall), bf16, kind="ExternalInput")
    al_d = nc.dram_tensor("alpha_out", (1, nall), f32, kind="ExternalOutput")
    co_d = nc.dram_tensor("color_out", (3, nall), f32, kind="ExternalOutput")

    with tile.TileContext(nc) as tc:
        with (
            tc.tile_pool(name="cp", bufs=1) as cp,
            tc.tile_pool(name="xp", bufs=2 * nw + 2) as xp,
            tc.tile_pool(name="psA", bufs=6, space=bass.MemorySpace.PSUM) as psA,
            tc.tile_pool(name="psB", bufs=2, space=bass.MemorySpace.PSUM) as psB,
        ):
            # ---- small inputs first (tiny, both HWDGE rings) ----
            # rows 0:53 for even slots, mirrored at 64:117 for odd slots
            # (matmul requires lhsT and rhs at the same partition base)
            embP = cp.tile([117, nall], bf16)  # sin 0:18, cos 32:50, xyz 50:53
            embD = cp.tile([111, nall], bf16)  # sin 0:12, cos 32:44, xyz 44:47
            nc.vector.memset(embP[:], 0.0)
            nc.vector.memset(embD[:], 0.0)
            fx_sb = cp.tile([3, 18], f32)
            nc.sync.dma_start(fx_sb[:], fx_d.ap()[:])
            fd_sb = cp.tile([3, 12], f32)
            nc.scalar.dma_start(fd_sb[:], fd_d.ap()[:])
            pts_sb = cp.tile([3, nall], f32)
            nc.sync.dma_start(pts_sb[:], pts_d.ap()[:])
            dir_sb = cp.tile([3, nall], f32)
            nc.scalar.dma_start(dir_sb[:], dir_d.ap()[:])
            bt_sb = cp.tile([128, NB * nslot], f32)
            nc.sync.dma_start(bt_sb[:], bt_d.ap()[:])
            # xyz rows arrive as pre-cast bf16 (plain HWDGE DMAs)
            nc.sync.dma_start(embP[50:53, :], xyz_d.ap()[0:3, :])
            nc.scalar.dma_start(embD[44:47, :], xyz_d.ap()[3:6, :])

            # ---- layer-major weight chunk DMAs over the 2 HWDGE rings;
            # arrival tracks consumption order, chunk 10 lands last ----
            RING0 = ["P", 2, 4, 6, 8]      # sync:   7.96MB
            RING1 = [1, 3, 5, 7, 10]       # scalar: 7.66MB
            wts = {}
            for st in CHUNKS:
                ncols = npair * P_PAIR_COLS if st == "P" else nslot * CHUNK_COLS[st]
                wts[st] = cp.tile([128, ncols], bf16, name=f"wt{st}", tag=f"wt{st}")
            for r, ring in enumerate((RING0, RING1)):
                eng = nc.sync if r == 0 else nc.scalar
                for st in ring:
                    ncols = npair * P_PAIR_COLS if st == "P" else nslot * CHUNK_COLS[st]
                    eng.dma_start(wts[st][:], wt_d.ap()[:, coff[st]: coff[st] + ncols])

            def slab(st, s, lo, hi, rows=128):
                o = s * CHUNK_COLS[st]
                return wts[st][0:rows, o + lo: o + hi]

            def pslab(kind, s):
                """paired-chunk slab for slot s: kind in (l0, skip, rays)."""
                rb = 64 * (s % 2) if PAIRED else 0
                pair = s // 2 if PAIRED else s
                if kind == "l0":
                    o, w, rows = pair * 256, 256, 53
                elif kind == "skip":
                    o, w, rows = npair * 256 + pair * 256, 256, 53
                else:
                    o, w, rows = npair * 512 + pair * 128, 128, 47
                return wts["P"][rb: rb + rows, o: o + w], w

            alpha_sb = cp.tile([1, nall], f32)
            color_sb = cp.tile([3, nall], f32)

            # frequency expansion + range-reduced sin/cos, in <=512-col chunks.
            def reduce_sin(tsrc, rows, ncol):
                t1 = xp.tile([rows, ncol], f32, tag="vred")
                nc.vector.tensor_scalar(t1[:], tsrc, INV2PI, MAGIC, MUL, ADD)
                r = xp.tile([rows, ncol], f32, tag="vred")
                nc.vector.tensor_scalar(r[:], t1[:], MAGIC, None, SUB)
                rd = xp.tile([rows, ncol], f32, tag="vred")
                nc.vector.scalar_tensor_tensor(rd[:], r[:], -C1, tsrc, MUL, ADD)
                rd2 = xp.tile([rows, ncol], f32, tag="vred")
                nc.vector.scalar_tensor_tensor(rd2[:], r[:], -C2, rd[:], MUL, ADD)
                v = xp.tile([rows, ncol], f32, tag="vred")
                nc.vector.tensor_scalar(v[:], rd2[:], CLAMP, -CLAMP, MIN, MAX)
                return v

            for lo in range(0, nall, 512):
                hi = min(nall, lo + 512)
                w_ = hi - lo
                for (rows, fmat, src, dst) in (
                    (18, fx_sb, pts_sb, embP),
                    (12, fd_sb, dir_sb, embD),
                ):
                    ep = psA.tile([rows, w_], f32, tag="mlp")
                    nc.tensor.matmul(ep[:], fmat[:, 0:rows], src[:, lo:hi],
                                     start=True, stop=True)
                    vs = reduce_sin(ep[:], rows, w_)
                    nc.scalar.activation(dst[0:rows, lo:hi], vs[:], Sin)
                    pre = xp.tile([rows, w_], f32, tag="vred")
                    nc.vector.tensor_scalar(pre[:], ep[:], HALF_PI, None, ADD)
                    vc = reduce_sin(pre[:], rows, w_)
                    cs = xp.tile([rows, w_], f32, tag="vred")
                    nc.scalar.activation(cs[:], vc[:], Sin)
                    nc.vector.tensor_copy(dst[32:32 + rows, lo:hi], cs[:])
            # mirror for odd slots (paired slabs at partition base 64); copies
            # must be <=32 partitions with 32-aligned sources, xyz via SWDGE
            if PAIRED:
                nc.vector.tensor_copy(embP[64:82, :], embP[0:18, :])
                nc.vector.tensor_copy(embP[96:114, :], embP[32:50, :])
                nc.sync.dma_start(embP[114:117, :], xyz_d.ap()[0:3, :])
                nc.vector.tensor_copy(embD[64:76, :], embD[0:12, :])
                nc.vector.tensor_copy(embD[96:108, :], embD[32:44, :])
                nc.scalar.dma_start(embD[108:111, :], xyz_d.ap()[3:6, :])

            # ---- wave-lockstep MLP ----
            def bias2_bcast(lidx, s0, s1):
                g = s1 - s0
                ap = bt_sb[:, lidx * 2 * nslot + s0 * 2: lidx * 2 * nslot + s1 * 2]
                return ap.rearrange("p (g j) -> p j g", j=2).broadcast_to(
                    [128, 2, g, C])

            def bias1_bcast(which, s0, s1, p=128):
                g = s1 - s0
                ap = bt_sb[0:p, which * nslot + s0: which * nslot + s1]
                return ap.broadcast_to([p, g, C])

            xs = [None] * nw
            its = [None] * nw
            cts = [None] * nw

            def mm_mid(st, ps, xin, s0, s1):
                for i in range(s1 - s0):
                    s = s0 + i
                    for j in (0, 1):
                        pj = ps[:, j, i * C:(i + 1) * C]
                        nc.tensor.matmul(pj, slab(st, s, j * 128, j * 128 + 128),
                                         xin[:, 0, i * C:(i + 1) * C],
                                         start=True, stop=False)
                        nc.tensor.matmul(pj, slab(st, s, 256 + j * 128, 256 + j * 128 + 128),
                                         xin[:, 1, i * C:(i + 1) * C],
                                         start=False, stop=True)

            def move2(ps, lidx, s0, s1, relu=True):
                g = s1 - s0
                xn = xp.tile([128, 2, g * C], bf16, tag="x")
                psv = ps[:].rearrange("p j (g c) -> p j g c", g=g)
                xnv = xn[:].rearrange("p j (g c) -> p j g c", g=g)
                nc.vector.tensor_tensor(xnv, psv, bias2_bcast(lidx, s0, s1), ADD)
                if relu:
                    nc.scalar.activation(xn[:], xn[:], Relu)
                return xn

            def emit_stage(wi_, stage):
                s0, s1 = waves[wi_]
                g = s1 - s0
                if stage == 0:  # L0
                    ps = psA.tile([128, 2, g * C], f32, tag="mlp")
                    for i in range(g):
                        s = s0 + i
                        sl = slice(s * C, (s + 1) * C)
                        rb = 64 * (s % 2) if PAIRED else 0
                        for j in (0, 1):
                            w0s, _ = pslab("l0", s)
                            nc.tensor.matmul(ps[:, j, i * C:(i + 1) * C],
                                             w0s[:, j * 128: j * 128 + 128],
                                             embP[rb: rb + 53, sl],
                                             start=True, stop=True)
                    xs[wi_] = move2(ps, 0, s0, s1)
                elif stage in (1, 2, 3, 4, 6, 7):
                    ps = psA.tile([128, 2, g * C], f32, tag="mlp")
                    mm_mid(stage, ps, xs[wi_], s0, s1)
                    xs[wi_] = move2(ps, stage, s0, s1)
                elif stage == 5:
                    ps = psA.tile([128, 2, g * C], f32, tag="mlp")
                    xin = xs[wi_]
                    for i in range(g):
                        s = s0 + i
                        sl = slice(s * C, (s + 1) * C)
                        for j in (0, 1):
                            pj = ps[:, j, i * C:(i + 1) * C]
                            nc.tensor.matmul(pj, slab(5, s, j * 128, j * 128 + 128),
                                             xin[:, 0, i * C:(i + 1) * C],
                                             start=True, stop=False)
                            nc.tensor.matmul(pj, slab(5, s, 256 + j * 128, 256 + j * 128 + 128),
                                             xin[:, 1, i * C:(i + 1) * C],
                                             start=False, stop=False)
                            w5s, _ = pslab("skip", s)
                            rb = 64 * (s % 2) if PAIRED else 0
                            nc.tensor.matmul(pj, w5s[:, j * 128: j * 128 + 128],
                                             embP[rb: rb + 53, sl],
                                             start=False, stop=True)
                    xs[wi_] = move2(ps, 5, s0, s1)
                elif stage == 8:  # wi -> inter (bias, no relu)
                    ps = psA.tile([128, 2, g * C], f32, tag="mlp")
                    mm_mid(8, ps, xs[wi_], s0, s1)
                    its[wi_] = move2(ps, 8, s0, s1, relu=False)
                elif stage == 9:  # wa -> alpha (weights ride in chunk 8)
                    pa = psB.tile([3, g * C], f32, tag="head")
                    xin = xs[wi_]
                    for i in range(g):
                        s = s0 + i
                        nc.tensor.matmul(pa[0:1, i * C:(i + 1) * C],
                                         slab(8, s, 512, 513),
                                         xin[:, 0, i * C:(i + 1) * C],
                                         start=True, stop=False)
                        nc.tensor.matmul(pa[0:1, i * C:(i + 1) * C],
                                         slab(8, s, 513, 514),
                                         xin[:, 1, i * C:(i + 1) * C],
                                         start=False, stop=True)
                    av = alpha_sb[0:1, s0 * C: s1 * C].rearrange(
                        "p (g c) -> p g c", g=g)
                    pav = pa[0:1, :].rearrange("p (g c) -> p g c", g=g)
                    nc.vector.tensor_tensor(av, pav, bias1_bcast(18, s0, s1, p=1), ADD)
                    nc.sync.dma_start(al_d.ap()[0:1, s0 * C: s1 * C],
                                      alpha_sb[0:1, s0 * C: s1 * C])
                elif stage == 10:  # wc0 -> c (relu)
                    pc = psA.tile([128, g * C], f32, tag="mlp")
                    it = its[wi_]
                    for i in range(g):
                        s = s0 + i
                        sl = slice(s * C, (s + 1) * C)
                        pj = pc[:, i * C:(i + 1) * C]
                        nc.tensor.matmul(pj, slab(10, s, 0, 128),
                                         it[:, 0, i * C:(i + 1) * C],
                                         start=True, stop=False)
                        nc.tensor.matmul(pj, slab(10, s, 128, 256),
                                         it[:, 1, i * C:(i + 1) * C],
                                         start=False, stop=False)
                        rays, _ = pslab("rays", s)
                        rb = 64 * (s % 2) if PAIRED else 0
                        nc.tensor.matmul(pj, rays[:], embD[rb: rb + 47, sl],
                                         start=False, stop=True)
                    ct = xp.tile([128, g * C], bf16, tag="ct")
                    pcv = pc[:].rearrange("p (g c) -> p g c", g=g)
                    ctv = ct[:].rearrange("p (g c) -> p g c", g=g)
                    nc.vector.tensor_tensor(ctv, pcv, bias1_bcast(19, s0, s1), ADD)
                    nc.scalar.activation(ct[:], ct[:], Relu)
                    cts[wi_] = ct
                elif stage == 11:  # wc1 -> sigmoid color (weights in chunk 10)
                    pcol = psB.tile([3, g * C], f32, tag="head")
                    ct = cts[wi_]
                    for i in range(g):
                        s = s0 + i
                        nc.tensor.matmul(pcol[:, i * C:(i + 1) * C],
                                         slab(10, s, 256, 259),
                                         ct[:, i * C:(i + 1) * C],
                                         start=True, stop=True)
                    ctmp = xp.tile([3, g * C], f32, tag="ctmp")
                    pv = pcol[:].rearrange("p (g c) -> p g c", g=g)
                    cv = ctmp[:].rearrange("p (g c) -> p g c", g=g)
                    nc.vector.tensor_tensor(cv, pv, bias1_bcast(20, s0, s1, p=3), ADD)
                    nc.scalar.activation(color_sb[0:3, s0 * C: s1 * C], ctmp[:],
                                         Sigmoid)
                    nc.scalar.dma_start(co_d.ap()[0:3, s0 * C: s1 * C],
                                        color_sb[0:3, s0 * C: s1 * C])

            for stage in range(12):
                for wi_ in range(nw):
                    emit_stage(wi_, stage)

    nc.compile()
    return nc


_prog_cache = {}
_last_results = None


def _get_program(C, nslot):
    key = (C, nslot)
    if key not in _prog_cache:
        _prog_cache[key] = _build_program(C, nslot)
    return _prog_cache[key]


# ---------------------------------------------------------------------------
# Host wrapper
# ---------------------------------------------------------------------------

def kernel(**inputs):
    global _last_results
    inputs = {k: np.asarray(v) for k, v in inputs.items()}
    idx = inputs["index"].astype(np.int64)
    B = idx.shape[0]
    points = inputs["points"].astype(np.float32)
    dirs = inputs["directions"].astype(np.float32)

    # --- routing: split each expert's tokens into <=CAP_MAX chunks ("virtual
    # experts"), distribute round-robin (sorted by size) over 8 cores ---
    tok = [np.nonzero(idx == e)[0] for e in range(E)]
    virt = []  # (expert, token_ids)
    for e in range(E):
        t = tok[e]
        if len(t) == 0:
            continue
        for lo in range(0, len(t), CAP_MAX):
            virt.append((e, t[lo: lo + CAP_MAX]))
    if not virt:
        virt = [(0, np.zeros((0,), np.int64))]
    virt.sort(key=lambda v: -len(v[1]))
    nslot = max(1, int(np.ceil(len(virt) / N_CORES)))
    C = max(4, int(np.ceil(max(len(v[1]) for v in virt) / 4) * 4))
    nall = nslot * C

    core_slots = [[] for _ in range(N_CORES)]
    for i, v in enumerate(virt):
        core_slots[i % N_CORES].append(v)

    nc = _get_program(C, nslot)
    coff, totcols = _chunk_offsets(nslot)

    fx = np.zeros((3, 18), np.float32)
    for c in range(3):
        for k in range(NX):
            fx[c, c * NX + k] = float(2 ** k)
    fd = np.zeros((3, 12), np.float32)
    for c in range(3):
        for k in range(ND):
            fd[c, c * ND + k] = float(2 ** k)

    in_maps = []
    for c in range(N_CORES):
        wt = np.zeros((128, totcols), np.float32)
        bt = np.zeros((128, NB * nslot), np.float32)
        ptsT = np.zeros((3, nall), np.float32)
        dirT = np.zeros((3, nall), np.float32)
        for s, (e, t) in enumerate(core_slots[c]):
            _pack_expert(wt, bt, s, nslot, inputs, e, coff)
            n = len(t)
            if n:
                ptsT[:, s * C: s * C + n] = points[t].T
                dirT[:, s * C: s * C + n] = dirs[t].T
        xyzb = np.concatenate([ptsT, dirT], axis=0).astype(ml_dtypes.bfloat16)
        in_maps.append({"wt": wt.astype(ml_dtypes.bfloat16), "bt": bt,
                        "ptsT": ptsT, "dirT": dirT, "fx": fx, "fd": fd,
                        "xyzb": xyzb})

    res = run_bass_kernel_spmd(nc, in_maps, core_ids=list(range(N_CORES)))
    _last_results = res

    out = np.zeros((B, 4), np.float32)
    for c in range(N_CORES):
        al = res.results[c]["alpha_out"]
        co = res.results[c]["color_out"]
        for s, (e, t) in enumerate(core_slots[c]):
            n = len(t)
            if n:
                out[t, 0] = al[0, s * C: s * C + n]
                out[t, 1:4] = co[:, s * C: s * C + n].T
    return out


# revision 28
# speedup vs baseline: 1.1045x; 1.1045x over previous
"""NerfExperts MoE kernel for Trainium2, expert-parallel over 8 NeuronCores.

Strategy: each of the 1024 points is routed to one of 100 experts
(~2.3MB of fp32 weights each, ~232MB total -> memory bound).  We shard
the *experts* across the 8 cores (13 slots per core), dispatch tokens to
their expert's core on the host, and stream each expert's weights from
HBM exactly once, as bf16.  Weights are streamed LAYER-MAJOR (one DMA
chunk per layer covering all local experts, balanced across the two
HWDGE rings) so compute for layer l only waits on chunk l and the
DMA/compute pipeline drains with a short tail.  Short slabs (the 53-row
embedding-consuming ones) are packed two experts per 128 partitions.
Activations stay transposed ([feature, token]); experts advance through
the MLP in lockstep "waves" that share PSUM tiles, so PSUM->SBUF
bias+activation moves are batched across a wave (per-expert fp32 biases
via stride-0 broadcast APs on DVE, relu on ACT).  Harmonic-embedding
phases are computed in fp32 with Cody-Waite range reduction for Sin.

Embedding tile rows: points: sin 0:18, cos 32:50, xyz 50:53 (53 rows);
dirs: sin 0:12, cos 32:44, xyz 44:47 (47 rows).  Dead rows are zero in
the weight slabs, so garbage there is harmless; paired slabs use row
base 64 for the odd expert of each pair.
"""

import numpy as np
import ml_dtypes

import concourse.bass as bass
import concourse.bacc as bacc
import concourse.mybir as mybir
import concourse.tile as tile
from concourse.bass_utils import run_bass_kernel_spmd

PI = float(np.pi)
N_CORES = 8
E = 100
NX, ND = 6, 4
CAP_MAX = 128  # max tokens per expert slot (keeps matmul N and PSUM in range)

# Weight chunks (key = mlp stage fed; "P" carries the paired short slabs:
# L0, L5-skip, wc0-rays; stage 9 (wa) rides in chunk 8, wc1 in chunk 10).
CHUNKS = ["P", 1, 2, 3, 4, 5, 6, 7, 8, 10]
CHUNK_COLS = {1: 512, 2: 512, 3: 512, 4: 512, 5: 512,
              6: 512, 7: 512, 8: 514, 10: 259}
P_PAIR_COLS = 640          # per expert PAIR: L0 256 + L5skip 256 + rays 128

# fp32 bias tensor [128, 21*nslot], layer-major columns:
#   mlp stage lidx in 0..8 (layers 0-7, then wi): col = lidx*2*nslot + s*2 + j
#   ba: 18*nslot + s ; bc0: 19*nslot + s ; bc1: 20*nslot + s
NB = 21
PAIRED = False  # base-64 paired slabs hang real HW (sim-clean); keep full-height slabs


def _chunk_offsets(nslot):
    npair = (nslot + 1) // 2 if PAIRED else nslot
    coff, tot = {}, 0
    coff["P"] = 0
    tot += npair * P_PAIR_COLS
    for st in CHUNKS[1:]:
        coff[st] = tot
        tot += nslot * CHUNK_COLS[st]
    return coff, tot


def _pack_expert(wt, bt, s, nslot, inputs, e, coff):
    """Fill slot s columns of the per-stage blocks of wt [128, TOT] (fp32 view)
    and bias columns of bt [128, 21*nslot]."""
    n2 = 2 * nslot
    npair = (nslot + 1) // 2 if PAIRED else nslot
    rb = 64 * (s % 2) if PAIRED else 0   # row base in paired chunk
    pair = s // 2 if PAIRED else s

    def set_b2(lidx, b):
        bt[:, lidx * n2 + s * 2] = b[0:128]
        bt[:, lidx * n2 + s * 2 + 1] = b[128:256]

    # --- paired chunk P ---
    o = coff["P"] + pair * 256
    w0 = inputs["w0"][e]                             # [39, 256]
    wt[rb + 0: rb + 18, o: o + 256] = w0[0:18]       # sin
    wt[rb + 32: rb + 50, o: o + 256] = w0[18:36]     # cos
    wt[rb + 50: rb + 53, o: o + 256] = w0[36:39]     # xyz
    set_b2(0, inputs["b0"][e])
    o = coff["P"] + npair * 256 + pair * 256
    w5 = inputs["w5"][e]                             # [295, 256]
    wt[rb + 0: rb + 18, o: o + 256] = w5[256:274]
    wt[rb + 32: rb + 50, o: o + 256] = w5[274:292]
    wt[rb + 50: rb + 53, o: o + 256] = w5[292:295]
    o = coff["P"] + npair * 512 + pair * 128
    wc0 = inputs["wc0"][e]                           # [283, 128]
    wt[rb + 0: rb + 12, o: o + 128] = wc0[256:268]
    wt[rb + 32: rb + 44, o: o + 128] = wc0[268:280]
    wt[rb + 44: rb + 47, o: o + 128] = wc0[280:283]
    # --- full-height chunks ---
    for l in (1, 2, 3, 4, 6, 7):
        w = inputs[f"w{l}"][e]                       # [256, 256]
        o = coff[l] + s * 512
        for k in (0, 1):
            wt[:, o + k * 256: o + (k + 1) * 256] = w[128 * k: 128 * (k + 1)]
        set_b2(l, inputs[f"b{l}"][e])
    o = coff[5] + s * 512
    for k in (0, 1):
        wt[:, o + k * 256: o + (k + 1) * 256] = w5[128 * k: 128 * (k + 1)]
    set_b2(5, inputs["b5"][e])
    o = coff[8] + s * 514
    wi = inputs["wi"][e]
    for k in (0, 1):
        wt[:, o + k * 256: o + (k + 1) * 256] = wi[128 * k: 128 * (k + 1)]
    set_b2(8, inputs["bi"][e])
    wa = inputs["wa"][e][:, 0]                       # [256]
    wt[:, o + 512] = wa[0:128]
    wt[:, o + 513] = wa[128:256]
    bt[0, 18 * nslot + s] = inputs["ba"][e][0]
    o = coff[10] + s * 259
    wt[:, o: o + 128] = wc0[0:128]
    wt[:, o + 128: o + 256] = wc0[128:256]
    bt[:, 19 * nslot + s] = inputs["bc0"][e]
    wt[:, o + 256: o + 259] = inputs["wc1"][e]
    bt[0:3, 20 * nslot + s] = inputs["bc1"][e]


def _make_waves(nslot, C):
    gmax = max(1, min(512 // (2 * C), 6))
    nw = int(np.ceil(nslot / gmax))
    base = nslot // nw
    rem = nslot - base * nw
    sizes = [base + (1 if i < rem else 0) for i in range(nw)]
    waves, s0 = [], 0
    for g in sizes:
        waves.append((s0, s0 + g))
        s0 += g
    return waves


# ---------------------------------------------------------------------------
# Device program
# ---------------------------------------------------------------------------

def _build_program(C, nslot):
    """Build the SPMD Bass program: nslot expert slots of C tokens each."""
    nall = nslot * C
    npair = (nslot + 1) // 2 if PAIRED else nslot
    waves = _make_waves(nslot, C)
    nw = len(waves)
    coff, totcols = _chunk_offsets(nslot)
    f32 = mybir.dt.float32
    bf16 = mybir.dt.bfloat16
    Sin = mybir.ActivationFunctionType.Sin
    Sigmoid = mybir.ActivationFunctionType.Sigmoid
    Relu = mybir.ActivationFunctionType.Relu
    ADD = mybir.AluOpType.add
    SUB = mybir.AluOpType.subtract
    MUL = mybir.AluOpType.mult
    MAX = mybir.AluOpType.max
    MIN = mybir.AluOpType.min
    # range-reduction constants (Cody-Waite, fp32 magic rounding)
    INV2PI = float(np.float32(1.0 / (2 * PI)))
    MAGIC = 12582912.0            # 1.5 * 2**23: forces round-to-int in fp32
    C1 = 6.28125                  # 2*pi high part, exact in fp32
    C2 = float(np.float32(2 * PI - 6.28125))
    CLAMP = 3.1415925             # just under pi (ACT Sin domain is [-pi, pi])
    HALF_PI = float(np.float32(PI / 2))

    nc = bacc.Bacc("TRN2", target_bir_lowering=False, debug=False)
    wt_d = nc.dram_tensor("wt", (128, totcols), bf16, kind="ExternalInput")
    bt_d = nc.dram_tensor("bt", (128, NB * nslot), f32, kind="ExternalInput")
    pts_d = nc.dram_tensor("ptsT", (3, nall), f32, kind="ExternalInput")
    dir_d = nc.dram_tensor("dirT", (3, nall), f32, kind="ExternalInput")
    fx_d = nc.dram_tensor("fx", (3, 18), f32, kind="ExternalInput")
    fd_d = nc.dram_tensor("fd", (3, 12), f32, kind="ExternalInput")
    xyz_d = nc.dram_tensor("xyzb", (6, nall), bf16, kind="ExternalInput")
    al_d = nc.dram_tensor("alpha_out", (1, nall), f32, kind="ExternalOutput")
    co_d = nc.dram_tensor("color_out", (3, nall), f32, kind="ExternalOutput")

    with tile.TileContext(nc) as tc:
        with (
            tc.tile_pool(name="cp", bufs=1) as cp,
            tc.tile_pool(name="xp", bufs=2 * nw + 2) as xp,
            tc.tile_pool(name="psA", bufs=6, space=bass.MemorySpace.PSUM) as psA,
            tc.tile_pool(name="psB", bufs=2, space=bass.MemorySpace.PSUM) as psB,
        ):
            # ---- small inputs first (tiny, both HWDGE rings) ----
            # rows 0:53 for even slots, mirrored at 64:117 for odd slots
            # (matmul requires lhsT and rhs at the same partition base)
            embP = cp.tile([117, nall], bf16)  # sin 0:18, cos 32:50, xyz 50:53
            embD = cp.tile([111, nall], bf16)  # sin 0:12, cos 32:44, xyz 44:47
            nc.vector.memset(embP[:], 0.0)
            nc.vector.memset(embD[:], 0.0)
            fx_sb = cp.tile([3, 18], f32)
            nc.sync.dma_start(fx_sb[:], fx_d.ap()[:])
            fd_sb = cp.tile([3, 12], f32)
            nc.scalar.dma_start(fd_sb[:], fd_d.ap()[:])
            pts_sb = cp.tile([3, nall], f32)
            nc.sync.dma_start(pts_sb[:], pts_d.ap()[:])
            dir_sb = cp.tile([3, nall], f32)
            nc.scalar.dma_start(dir_sb[:], dir_d.ap()[:])
            bt_sb = cp.tile([128, NB * nslot], f32)
            nc.sync.dma_start(bt_sb[:], bt_d.ap()[:])
            # xyz rows arrive as pre-cast bf16 (plain HWDGE DMAs)
            nc.sync.dma_start(embP[50:53, :], xyz_d.ap()[0:3, :])
            nc.scalar.dma_start(embD[44:47, :], xyz_d.ap()[3:6, :])

            # ---- layer-major weight chunk DMAs over the 2 HWDGE rings;
            # arrival tracks consumption order, chunk 10 lands last ----
            RING0 = ["P", 2, 4, 8]         # sync: c8 lands ~3us before c7
            RING1 = [1, 3, 5, 6, 7, 10]    # scalar: c10 lands last -> short tail
            wts = {}
            for st in CHUNKS:
                ncols = npair * P_PAIR_COLS if st == "P" else nslot * CHUNK_COLS[st]
                wts[st] = cp.tile([128, ncols], bf16, name=f"wt{st}", tag=f"wt{st}")
            for r, ring in enumerate((RING0, RING1)):
                eng = nc.sync if r == 0 else nc.scalar
                for st in ring:
                    ncols = npair * P_PAIR_COLS if st == "P" else nslot * CHUNK_COLS[st]
                    eng.dma_start(wts[st][:], wt_d.ap()[:, coff[st]: coff[st] + ncols])

            def slab(st, s, lo, hi, rows=128):
                o = s * CHUNK_COLS[st]
                return wts[st][0:rows, o + lo: o + hi]

            def pslab(kind, s):
                """paired-chunk slab for slot s: kind in (l0, skip, rays)."""
                rb = 64 * (s % 2) if PAIRED else 0
                pair = s // 2 if PAIRED else s
                if kind == "l0":
                    o, w, rows = pair * 256, 256, 53
                elif kind == "skip":
                    o, w, rows = npair * 256 + pair * 256, 256, 53
                else:
                    o, w, rows = npair * 512 + pair * 128, 128, 47
                return wts["P"][rb: rb + rows, o: o + w], w

            alpha_sb = cp.tile([1, nall], f32)
            color_sb = cp.tile([3, nall], f32)

            # frequency expansion + range-reduced sin/cos, in <=512-col chunks.
            def reduce_sin(tsrc, rows, ncol):
                t1 = xp.tile([rows, ncol], f32, tag="vred")
                nc.vector.tensor_scalar(t1[:], tsrc, INV2PI, MAGIC, MUL, ADD)
                r = xp.tile([rows, ncol], f32, tag="vred")
                nc.vector.tensor_scalar(r[:], t1[:], MAGIC, None, SUB)
                rd = xp.tile([rows, ncol], f32, tag="vred")
                nc.vector.scalar_tensor_tensor(rd[:], r[:], -C1, tsrc, MUL, ADD)
                rd2 = xp.tile([rows, ncol], f32, tag="vred")
                nc.vector.scalar_tensor_tensor(rd2[:], r[:], -C2, rd[:], MUL, ADD)
                v = xp.tile([rows, ncol], f32, tag="vred")
                nc.vector.tensor_scalar(v[:], rd2[:], CLAMP, -CLAMP, MIN, MAX)
                return v

            for lo in range(0, nall, 512):
                hi = min(nall, lo + 512)
                w_ = hi - lo
                for (rows, fmat, src, dst) in (
                    (18, fx_sb, pts_sb, embP),
                    (12, fd_sb, dir_sb, embD),
                ):
                    ep = psA.tile([rows, w_], f32, tag="mlp")
                    nc.tensor.matmul(ep[:], fmat[:, 0:rows], src[:, lo:hi],
                                     start=True, stop=True)
                    vs = reduce_sin(ep[:], rows, w_)
                    nc.scalar.activation(dst[0:rows, lo:hi], vs[:], Sin)
                    pre = xp.tile([rows, w_], f32, tag="vred")
                    nc.vector.tensor_scalar(pre[:], ep[:], HALF_PI, None, ADD)
                    vc = reduce_sin(pre[:], rows, w_)
                    cs = xp.tile([rows, w_], f32, tag="vred")
                    nc.scalar.activation(cs[:], vc[:], Sin)
                    nc.vector.tensor_copy(dst[32:32 + rows, lo:hi], cs[:])
            # mirror for odd slots (paired slabs at partition base 64); copies
            # must be <=32 partitions with 32-aligned sources, xyz via SWDGE
            if PAIRED:
                nc.vector.tensor_copy(embP[64:82, :], embP[0:18, :])
                nc.vector.tensor_copy(embP[96:114, :], embP[32:50, :])
                nc.sync.dma_start(embP[114:117, :], xyz_d.ap()[0:3, :])
                nc.vector.tensor_copy(embD[64:76, :], embD[0:12, :])
                nc.vector.tensor_copy(embD[96:108, :], embD[32:44, :])
                nc.scalar.dma_start(embD[108:111, :], xyz_d.ap()[3:6, :])

            # ---- wave-lockstep MLP ----
            def bias2_bcast(lidx, s0, s1):
                g = s1 - s0
                ap = bt_sb[:, lidx * 2 * nslot + s0 * 2: lidx * 2 * nslot + s1 * 2]
                return ap.rearrange("p (g j) -> p j g", j=2).broadcast_to(
                    [128, 2, g, C])

            def bias1_bcast(which, s0, s1, p=128):
                g = s1 - s0
                ap = bt_sb[0:p, which * nslot + s0: which * nslot + s1]
                return ap.broadcast_to([p, g, C])

            xs = [None] * nw
            its = [None] * nw
            cts = [None] * nw

            def mm_mid(st, ps, xin, s0, s1):
                for i in range(s1 - s0):
                    s = s0 + i
                    for j in (0, 1):
                        pj = ps[:, j, i * C:(i + 1) * C]
                        nc.tensor.matmul(pj, slab(st, s, j * 128, j * 128 + 128),
                                         xin[:, 0, i * C:(i + 1) * C],
                                         start=True, stop=False)
                        nc.tensor.matmul(pj, slab(st, s, 256 + j * 128, 256 + j * 128 + 128),
                                         xin[:, 1, i * C:(i + 1) * C],
                                         start=False, stop=True)

            def move2(ps, lidx, s0, s1, relu=True):
                g = s1 - s0
                xn = xp.tile([128, 2, g * C], bf16, tag="x")
                psv = ps[:].rearrange("p j (g c) -> p j g c", g=g)
                xnv = xn[:].rearrange("p j (g c) -> p j g c", g=g)
                nc.vector.tensor_tensor(xnv, psv, bias2_bcast(lidx, s0, s1), ADD)
                if relu:
                    nc.scalar.activation(xn[:], xn[:], Relu)
                return xn

            def emit_stage(wi_, stage):
                s0, s1 = waves[wi_]
                g = s1 - s0
                if stage == 0:  # L0
                    ps = psA.tile([128, 2, g * C], f32, tag="mlp")
                    for i in range(g):
                        s = s0 + i
                        sl = slice(s * C, (s + 1) * C)
                        rb = 64 * (s % 2) if PAIRED else 0
                        for j in (0, 1):
                            w0s, _ = pslab("l0", s)
                            nc.tensor.matmul(ps[:, j, i * C:(i + 1) * C],
                                             w0s[:, j * 128: j * 128 + 128],
                                             embP[rb: rb + 53, sl],
                                             start=True, stop=True)
                    xs[wi_] = move2(ps, 0, s0, s1)
                elif stage in (1, 2, 3, 4, 6, 7):
                    ps = psA.tile([128, 2, g * C], f32, tag="mlp")
                    mm_mid(stage, ps, xs[wi_], s0, s1)
                    xs[wi_] = move2(ps, stage, s0, s1)
                elif stage == 5:
                    ps = psA.tile([128, 2, g * C], f32, tag="mlp")
                    xin = xs[wi_]
                    for i in range(g):
                        s = s0 + i
                        sl = slice(s * C, (s + 1) * C)
                        for j in (0, 1):
                            pj = ps[:, j, i * C:(i + 1) * C]
                            nc.tensor.matmul(pj, slab(5, s, j * 128, j * 128 + 128),
                                             xin[:, 0, i * C:(i + 1) * C],
                                             start=True, stop=False)
                            nc.tensor.matmul(pj, slab(5, s, 256 + j * 128, 256 + j * 128 + 128),
                                             xin[:, 1, i * C:(i + 1) * C],
                                             start=False, stop=False)
                            w5s, _ = pslab("skip", s)
                            rb = 64 * (s % 2) if PAIRED else 0
                            nc.tensor.matmul(pj, w5s[:, j * 128: j * 128 + 128],
                                             embP[rb: rb + 53, sl],
                                             start=False, stop=True)
                    xs[wi_] = move2(ps, 5, s0, s1)
                elif stage == 8:  # wi -> inter (bias, no relu)
                    ps = psA.tile([128, 2, g * C], f32, tag="mlp")
                    mm_mid(8, ps, xs[wi_], s0, s1)
                    its[wi_] = move2(ps, 8, s0, s1, relu=False)
                elif stage == 9:  # wa -> alpha (weights ride in chunk 8)
                    pa = psB.tile([3, g * C], f32, tag="head")
                    xin = xs[wi_]
                    for i in range(g):
                        s = s0 + i
                        nc.tensor.matmul(pa[0:1, i * C:(i + 1) * C],
                                         slab(8, s, 512, 513),
                                         xin[:, 0, i * C:(i + 1) * C],
                                         start=True, stop=False)
                        nc.tensor.matmul(pa[0:1, i * C:(i + 1) * C],
                                         slab(8, s, 513, 514),
                                         xin[:, 1, i * C:(i + 1) * C],
                                         start=False, stop=True)
                    av = alpha_sb[0:1, s0 * C: s1 * C].rearrange(
                        "p (g c) -> p g c", g=g)
                    pav = pa[0:1, :].rearrange("p (g c) -> p g c", g=g)
                    nc.vector.tensor_tensor(av, pav, bias1_bcast(18, s0, s1, p=1), ADD)
                elif stage == 10:  # wc0 -> c (relu)
                    pc = psA.tile([128, g * C], f32, tag="mlp")
                    it = its[wi_]
                    for i in range(g):
                        s = s0 + i
                        sl = slice(s * C, (s + 1) * C)
                        pj = pc[:, i * C:(i + 1) * C]
                        nc.tensor.matmul(pj, slab(10, s, 0, 128),
                                         it[:, 0, i * C:(i + 1) * C],
                                         start=True, stop=False)
                        nc.tensor.matmul(pj, slab(10, s, 128, 256),
                                         it[:, 1, i * C:(i + 1) * C],
                                         start=False, stop=False)
                        rays, _ = pslab("rays", s)
                        rb = 64 * (s % 2) if PAIRED else 0
                        nc.tensor.matmul(pj, rays[:], embD[rb: rb + 47, sl],
                                         start=False, stop=True)
                    ct = xp.tile([128, g * C], bf16, tag="ct")
                    pcv = pc[:].rearrange("p (g c) -> p g c", g=g)
                    ctv = ct[:].rearrange("p (g c) -> p g c", g=g)
                    nc.vector.tensor_tensor(ctv, pcv, bias1_bcast(19, s0, s1), ADD)
                    nc.scalar.activation(ct[:], ct[:], Relu)
                    cts[wi_] = ct
                elif stage == 11:  # wc1 -> sigmoid color (weights in chunk 10)
                    pcol = psB.tile([3, g * C], f32, tag="head")
                    ct = cts[wi_]
                    for i in range(g):
                        s = s0 + i
                        nc.tensor.matmul(pcol[:, i * C:(i + 1) * C],
                                         slab(10, s, 256, 259),
                                         ct[:, i * C:(i + 1) * C],
                                         start=True, stop=True)
                    ctmp = xp.tile([3, g * C], f32, tag="ctmp")
                    pv = pcol[:].rearrange("p (g c) -> p g c", g=g)
                    cv = ctmp[:].rearrange("p (g c) -> p g c", g=g)
                    nc.vector.tensor_tensor(cv, pv, bias1_bcast(20, s0, s1, p=3), ADD)
                    nc.scalar.activation(color_sb[0:3, s0 * C: s1 * C], ctmp[:],
                                         Sigmoid)

            for stage in range(12):
                for wi_ in range(nw):
                    emit_stage(wi_, stage)

            nc.sync.dma_start(al_d.ap()[:], alpha_sb[:])
            nc.sync.dma_start(co_d.ap()[:], color_sb[:])

    nc.compile()
    return nc


_prog_cache = {}
_last_results = None


def _get_program(C, nslot):
    key = (C, nslot)
    if key not in _prog_cache:
        _prog_cache[key] = _build_program(C, nslot)
    return _prog_cache[key]


# ---------------------------------------------------------------------------
# Host wrapper
# ---------------------------------------------------------------------------

def kernel(**inputs):
    global _last_results
    inputs = {k: np.asarray(v) for k, v in inputs.items()}
    idx = inputs["index"].astype(np.int64)
    B = idx.shape[0]
    points = inputs["points"].astype(np.float32)
    dirs = inputs["directions"].astype(np.float32)

    # --- routing: split each expert's tokens into <=CAP_MAX chunks ("virtual
    # experts"), distribute round-robin (sorted by size) over 8 cores ---
    tok = [np.nonzero(idx == e)[0] for e in range(E)]
    virt = []  # (expert, token_ids)
    for e in range(E):
        t = tok[e]
        if len(t) == 0:
            continue
        for lo in range(0, len(t), CAP_MAX):
            virt.append((e, t[lo: lo + CAP_MAX]))
    if not virt:
        virt = [(0, np.zeros((0,), np.int64))]
    virt.sort(key=lambda v: -len(v[1]))
    nslot = max(1, int(np.ceil(len(virt) / N_CORES)))
    C = max(4, int(np.ceil(max(len(v[1]) for v in virt) / 4) * 4))
    nall = nslot * C

    core_slots = [[] for _ in range(N_CORES)]
    for i, v in enumerate(virt):
        core_slots[i % N_CORES].append(v)

    nc = _get_program(C, nslot)
    coff, totcols = _chunk_offsets(nslot)

    fx = np.zeros((3, 18), np.float32)
    for c in range(3):
        for k in range(NX):
            fx[c, c * NX + k] = float(2 ** k)
    fd = np.zeros((3, 12), np.float32)
    for c in range(3):
        for k in range(ND):
            fd[c, c * ND + k] = float(2 ** k)

    in_maps = []
    for c in range(N_CORES):
        wt = np.zeros((128, totcols), np.float32)
        bt = np.zeros((128, NB * nslot), np.float32)
        ptsT = np.zeros((3, nall), np.float32)
        dirT = np.zeros((3, nall), np.float32)
        for s, (e, t) in enumerate(core_slots[c]):
            _pack_expert(wt, bt, s, nslot, inputs, e, coff)
            n = len(t)
            if n:
                ptsT[:, s * C: s * C + n] = points[t].T
                dirT[:, s * C: s * C + n] = dirs[t].T
        xyzb = np.concatenate([ptsT, dirT], axis=0).astype(ml_dtypes.bfloat16)
        in_maps.append({"wt": wt.astype(ml_dtypes.bfloat16), "bt": bt,
                        "ptsT": ptsT, "dirT": dirT, "fx": fx, "fd": fd,
                        "xyzb": xyzb})

    res = run_bass_kernel_spmd(nc, in_maps, core_ids=list(range(N_CORES)))
    _last_results = res

    out = np.zeros((B, 4), np.float32)
    for c in range(N_CORES):
        al = res.results[c]["alpha_out"]
        co = res.results[c]["color_out"]
        for s, (e, t) in enumerate(core_slots[c]):
            n = len(t)
            if n:
                out[t, 0] = al[0, s * C: s * C + n]
                out[t, 1:4] = co[:, s * C: s * C + n].T
    return out
